# revision 6
# baseline (speedup 1.0000x reference)
import sys

if "/opt/trn_rl_repo" not in sys.path:
    sys.path.insert(0, "/opt/trn_rl_repo")

from contextlib import ExitStack

import numpy as np

import concourse.bass as bass
import concourse.mybir as mybir
import concourse.tile as tile
from concourse import bacc, masks
from concourse.bass_utils import run_bass_kernel_spmd

F32 = mybir.dt.float32

N_CORES = 8
N_FULL = 512
M_FULL = 512
NSH = N_FULL // N_CORES
C_S, C_H, C_Z = 384, 128, 128
D = 0.001 + float(N_FULL * M_FULL)
LN_EPS = 1e-5
SUP = 8
MMG = 4


def _build_program() -> bass.Bass:
    nc = bacc.Bacc("TRN2", target_bir_lowering=False)

    s1c = nc.declare_dram_parameter("s1c", [NSH, C_S], F32, isOutput=False)
    s2 = nc.declare_dram_parameter("s2", [M_FULL, C_S], F32, isOutput=False)
    W1 = nc.declare_dram_parameter("W1", [C_H, C_S], F32, isOutput=False)
    W2 = nc.declare_dram_parameter("W2", [C_H, C_S], F32, isOutput=False)
    Wout = nc.declare_dram_parameter("Wout", [C_Z, 2 * C_H], F32, isOutput=False)
    b1v = nc.declare_dram_parameter("b1v", [1, C_H], F32, isOutput=False)
    b2v = nc.declare_dram_parameter("b2v", [1, C_H], F32, isOutput=False)
    gammav = nc.declare_dram_parameter("gammav", [1, C_H], F32, isOutput=False)
    betav = nc.declare_dram_parameter("betav", [1, C_H], F32, isOutput=False)
    boutv = nc.declare_dram_parameter("boutv", [1, C_Z], F32, isOutput=False)
    out = nc.declare_dram_parameter("out", [NSH, M_FULL, C_Z], F32, isOutput=True)

    out_r = out[:].rearrange("n (p c) z -> p n (c z)", p=128, c=4)

    with tile.TileContext(nc) as tc, ExitStack() as ctx:
        const = ctx.enter_context(tc.tile_pool(name="const", bufs=1))
        wpool = ctx.enter_context(tc.tile_pool(name="wpool", bufs=1))
        work = ctx.enter_context(tc.tile_pool(name="work", bufs=2))
        small = ctx.enter_context(tc.tile_pool(name="small", bufs=3))
        stage_pool = ctx.enter_context(tc.tile_pool(name="stage", bufs=3))
        drampool = ctx.enter_context(tc.tile_pool(name="dram", bufs=1, space="DRAM"))
        pspool = ctx.enter_context(tc.tile_pool(name="ps", bufs=3, space="PSUM"))
        psout = ctx.enter_context(tc.tile_pool(name="psout", bufs=MMG, space="PSUM"))

        I128 = const.tile([128, 128], F32)
        masks.make_identity(nc, I128[:])
        ones_row = const.tile([1, 128], F32)
        nc.gpsimd.memset(ones_row[:], 1.0)
        ones_col = const.tile([128, 1], F32)
        nc.gpsimd.memset(ones_col[:], 1.0)

        W1s = wpool.tile([C_H, C_S], F32)
        nc.gpsimd.dma_start(W1s[:], W1[:])
        W2s = wpool.tile([C_H, C_S], F32)
        nc.gpsimd.dma_start(W2s[:], W2[:])
        Wouts = wpool.tile([C_Z, 2 * C_H], F32)
        nc.gpsimd.dma_start(Wouts[:], Wout[:])

        vrows = {}
        for name, src in (("b1", b1v), ("b2", b2v), ("gamma", gammav),
                          ("beta", betav), ("bout", boutv)):
            t = const.tile([1, 128], F32, tag=f"v_{name}")
            nc.gpsimd.dma_start(t[:], src[:])
            vrows[name] = t

        s1s = wpool.tile([NSH, C_S], F32)
        nc.gpsimd.dma_start(s1s[:], s1c[:])
        s2_re = s2[:].rearrange("(q four) s -> four q s", four=4)
        s2s = []
        for c in range(4):
            t = wpool.tile([128, C_S], F32, tag=f"s2_{c}")
            nc.gpsimd.dma_start(t[:], s2_re[c])
            s2s.append(t)

        W1T = wpool.tile([128, C_S], F32)
        W2T = wpool.tile([128, C_S], F32)
        for Wsrc, Wdst in ((W1s, W1T), (W2s, W2T)):
            for cs in range(3):
                pst = pspool.tile([128, 128], F32, tag="ps")
                nc.tensor.transpose(pst[:], Wsrc[:, cs * 128:(cs + 1) * 128], I128[:])
                nc.vector.tensor_copy(Wdst[:, cs * 128:(cs + 1) * 128], pst[:])

        WABTr = wpool.tile([128, 256], F32)
        for ch in range(2):
            pst = pspool.tile([128, 128], F32, tag="ps")
            nc.tensor.transpose(pst[:], Wouts[:, ch * 128:(ch + 1) * 128], I128[:])
            nc.vector.tensor_copy(WABTr[:, ch * 128:(ch + 1) * 128], pst[:])

        cols = {}
        for name in ("gamma", "beta"):
            pst = pspool.tile([128, 1], F32, tag="ps")
            nc.tensor.transpose(pst[:], vrows[name][:], I128[0:1, 0:1])
            t = const.tile([128, 1], F32, tag=f"c_{name}")
            nc.vector.tensor_copy(t[:], pst[:])
            cols[name] = t

        WABT = wpool.tile([128, 256], F32)
        nc.scalar.mul(WABT[:], WABTr[:], cols["gamma"][:, 0:1])

        psw = pspool.tile([1, 256], F32, tag="ps")
        nc.tensor.matmul(psw[0:1, 0:128], lhsT=ones_col[:], rhs=WABT[:, 0:128],
                         start=True, stop=True)
        nc.tensor.matmul(psw[0:1, 128:256], lhsT=ones_col[:], rhs=WABT[:, 128:256],
                         start=True, stop=True)
        wsum = const.tile([1, 256], F32)
        nc.vector.tensor_copy(wsum[:], psw[:])

        boutD = const.tile([1, 128], F32)
        nc.scalar.mul(boutD[:], vrows["bout"][:], float(D))
        psb = pspool.tile([1, 128], F32, tag="ps")
        nc.tensor.matmul(psb[:], lhsT=cols["beta"][:], rhs=WABTr[:, 0:128],
                         start=True, stop=False)
        nc.tensor.matmul(psb[:], lhsT=cols["beta"][:], rhs=WABTr[:, 128:256],
                         start=False, stop=False, skip_group_check=True)
        nc.tensor.matmul(psb[:], lhsT=ones_row[0:1, 0:1], rhs=boutD[:],
                         start=False, stop=True, skip_group_check=True)
        bias_row = const.tile([1, 128], F32)
        nc.vector.tensor_copy(bias_row[:], psb[:])

        def ln_stats(xT_sbuf, sq_sbuf, nn, tagp):
            pss = pspool.tile([1, 2 * nn], F32, tag="ps")
            nc.tensor.matmul(pss[0:1, 0:nn], lhsT=ones_col[:], rhs=xT_sbuf[:],
                             start=True, stop=True)
            nc.tensor.matmul(pss[0:1, nn:2 * nn], lhsT=ones_col[:], rhs=sq_sbuf[:],
                             start=True, stop=True)
            negmu = small.tile([1, nn], F32, tag=f"negmu{tagp}")
            nc.scalar.mul(negmu[:], pss[0:1, 0:nn], -1.0 / C_H)
            e2 = small.tile([1, nn], F32, tag=f"e2{tagp}")
            nc.scalar.activation(e2[:], pss[0:1, nn:2 * nn],
                                 mybir.ActivationFunctionType.Copy,
                                 bias=float(LN_EPS), scale=1.0 / C_H)
            musq = small.tile([1, nn], F32, tag=f"musq{tagp}")
            nc.vector.tensor_mul(musq[:], negmu[:], negmu[:])
            var = small.tile([1, nn], F32, tag=f"var{tagp}")
            nc.vector.tensor_sub(var[:], e2[:], musq[:])
            sd = small.tile([1, nn], F32, tag=f"sd{tagp}")
            nc.scalar.activation(sd[:], var[:],
                                 mybir.ActivationFunctionType.Sqrt)
            rstd_row = small.tile([1, nn], F32, tag=f"rstdr{tagp}")
            nc.vector.reciprocal(rstd_row[:], sd[:])
            pst2 = pspool.tile([nn, 1], F32, tag="ps")
            nc.tensor.transpose(pst2[:], rstd_row[:], I128[0:1, 0:1])
            rstd_col = small.tile([nn, 1], F32, tag=f"rstdc{tagp}")
            nc.vector.tensor_copy(rstd_col[:], pst2[:])
            return negmu, rstd_col

        s1T = wpool.tile([128, 3 * NSH], F32)
        for cs in range(3):
            pst = pspool.tile([128, NSH], F32, tag="ps")
            nc.tensor.transpose(pst[:], s1s[:, cs * 128:(cs + 1) * 128],
                                I128[0:NSH, 0:NSH])
            nc.vector.tensor_copy(s1T[:, cs * NSH:(cs + 1) * NSH], pst[:])

        psa = pspool.tile([128, NSH], F32, tag="ps")
        for cs in range(3):
            nc.tensor.matmul(psa[:], lhsT=W1T[:, cs * 128:(cs + 1) * 128],
                             rhs=s1T[:, cs * NSH:(cs + 1) * NSH],
                             start=(cs == 0), stop=False)
        nc.tensor.matmul(psa[:], lhsT=vrows["b1"][:], rhs=ones_row[0:1, 0:NSH],
                         start=False, stop=True, skip_group_check=True)
        apreT = work.tile([128, NSH], F32, tag="apreT")
        nc.vector.tensor_copy(apreT[:], psa[:])
        sqa = work.tile([128, NSH], F32, tag="sqa")
        nc.scalar.square(sqa[:], psa[:])

        negmu_a, rstd_a = ln_stats(apreT, sqa, NSH, "a")

        psp = pspool.tile([NSH, 128], F32, tag="ps")
        nc.tensor.matmul(psp[:], lhsT=apreT[:], rhs=WABT[:, 0:128],
                         start=True, stop=False)
        nc.tensor.matmul(psp[:], lhsT=negmu_a[:], rhs=wsum[0:1, 0:128],
                         start=False, stop=True, skip_group_check=True)
        pa_s = work.tile([NSH, 128], F32, tag="pa_s")
        nc.scalar.mul(pa_s[:], psp[:], rstd_a[:, 0:1])

        pa_dram = drampool.tile([NSH, 128], F32)
        nc.sync.dma_start(pa_dram[:], pa_s[:])
        pa_rep2 = wpool.tile([1, NSH * 256], F32)
        pa_rep2_v = pa_rep2[:].rearrange("a (n c z) -> a n c z", n=NSH, c=2)
        for c2 in range(2):
            nc.sync.dma_start(pa_rep2_v[:, :, c2:c2 + 1, :], pa_dram[:])

        pb_all = wpool.tile([128, 512], F32)
        for c in range(4):
            s2T_c = work.tile([128, C_S], F32, tag="s2T")
            for cs in range(3):
                pst = pspool.tile([128, 128], F32, tag="ps")
                nc.tensor.transpose(pst[:], s2s[c][:, cs * 128:(cs + 1) * 128],
                                    I128[:])
                nc.vector.tensor_copy(s2T_c[:, cs * 128:(cs + 1) * 128], pst[:])
            psb_ = pspool.tile([128, 128], F32, tag="ps")
            for cs in range(3):
                nc.tensor.matmul(psb_[:], lhsT=W2T[:, cs * 128:(cs + 1) * 128],
                                 rhs=s2T_c[:, cs * 128:(cs + 1) * 128],
                                 start=(cs == 0), stop=False)
            nc.tensor.matmul(psb_[:], lhsT=vrows["b2"][:], rhs=ones_row[:],
                             start=False, stop=True, skip_group_check=True)
            bpreT = work.tile([128, 128], F32, tag="bpreT")
            nc.vector.tensor_copy(bpreT[:], psb_[:])
            sqb = work.tile([128, 128], F32, tag="sqb")
            nc.scalar.square(sqb[:], psb_[:])

            negmu_b, rstd_b = ln_stats(bpreT, sqb, 128, "b")

            pspb = pspool.tile([128, 128], F32, tag="ps")
            nc.tensor.matmul(pspb[:], lhsT=bpreT[:], rhs=WABT[:, 128:256],
                             start=True, stop=False)
            nc.tensor.matmul(pspb[:], lhsT=negmu_b[:], rhs=wsum[0:1, 128:256],
                             start=False, stop=False, skip_group_check=True)
            nc.tensor.matmul(pspb[:], lhsT=ones_row[:], rhs=bias_row[:],
                             start=False, stop=True, skip_group_check=True)
            nc.scalar.mul(pb_all[:, c * 128:(c + 1) * 128], pspb[:],
                          rstd_b[:, 0:1])

        inv_d = 1.0 / D
        for g in range(NSH // SUP):
            stg = stage_pool.tile([128, SUP * 512], F32, tag="stage")
            for jg in range(0, SUP, MMG):
                pss = []
                for j in range(jg, jg + MMG):
                    ps = psout.tile([128, 512], F32, tag="ps_out")
                    nc.tensor.matmul(ps[:], lhsT=I128[:], rhs=pb_all[:],
                                     start=True, stop=False)
                    pss.append(ps)
                for idx, j in enumerate(range(jg, jg + MMG)):
                    n = g * SUP + j
                    ps = pss[idx]
                    rep = pa_rep2[0:1, n * 256:(n + 1) * 256]
                    nc.tensor.matmul(ps[:, 0:256], lhsT=ones_row[:], rhs=rep,
                                     start=False, stop=False,
                                     skip_group_check=True)
                    nc.tensor.matmul(ps[:, 256:512], lhsT=ones_row[:], rhs=rep,
                                     start=False, stop=True,
                                     skip_group_check=True)
                for idx, j in enumerate(range(jg, jg + MMG)):
                    dst = stg[:, j * 512:(j + 1) * 512]
                    if j % 2 == 0:
                        nc.scalar.mul(dst, pss[idx][:], inv_d)
                    else:
                        nc.vector.tensor_scalar_mul(dst, pss[idx][:], inv_d)
            nc.sync.dma_start(
                out_r[:, g * SUP:(g + 1) * SUP, :],
                stg[:].rearrange("p (n f) -> p n f", n=SUP),
            )

    nc.compile()
    return nc


_CACHE = {}


def _get_program() -> bass.Bass:
    if "nc" not in _CACHE:
        _CACHE["nc"] = _build_program()
    return _CACHE["nc"]


def _make_in_maps(inputs: dict) -> list[dict]:
    s1 = np.ascontiguousarray(np.asarray(inputs["s1"], dtype=np.float32))
    s2 = np.ascontiguousarray(np.asarray(inputs["s2"], dtype=np.float32))
    W1 = np.ascontiguousarray(np.asarray(inputs["W1"], dtype=np.float32))
    W2 = np.ascontiguousarray(np.asarray(inputs["W2"], dtype=np.float32))
    Wout = np.ascontiguousarray(np.asarray(inputs["Wout"], dtype=np.float32))
    row = lambda v: np.ascontiguousarray(
        np.asarray(v, dtype=np.float32).reshape(1, -1))
    shared = {
        "s2": s2[0],
        "W1": W1, "W2": W2, "Wout": Wout,
        "b1v": row(inputs["b1"]), "b2v": row(inputs["b2"]),
        "gammav": row(inputs["gamma"]), "betav": row(inputs["beta"]),
        "boutv": row(inputs["bout"]),
    }
    in_maps = []
    for i in range(N_CORES):
        m = dict(shared)
        m["s1c"] = np.ascontiguousarray(s1[0, i * NSH:(i + 1) * NSH, :])
        in_maps.append(m)
    return in_maps


def run(inputs: dict, **spmd_kwargs):
    nc = _get_program()
    in_maps = _make_in_maps(inputs)
    res = run_bass_kernel_spmd(nc, in_maps, list(range(N_CORES)), **spmd_kwargs)
    parts = [res.results[i]["out"] for i in range(N_CORES)]
    full = np.concatenate(parts, axis=0)[None]
    return full, res


def kernel(**inputs) -> np.ndarray:
    full, _ = run(inputs)
    return full


if __name__ == "__main__":
    rng = np.random.default_rng(0)
    fake = {
        "s1": rng.standard_normal((1, 512, 384), dtype=np.float32),
        "s2": rng.standard_normal((1, 512, 384), dtype=np.float32),
        "W1": rng.standard_normal((128, 384), dtype=np.float32) / np.sqrt(384),
        "b1": np.zeros(128, np.float32),
        "W2": rng.standard_normal((128, 384), dtype=np.float32) / np.sqrt(384),
        "b2": np.zeros(128, np.float32),
        "gamma": np.ones(128, np.float32),
        "beta": np.zeros(128, np.float32),
        "Wout": rng.standard_normal((128, 256), dtype=np.float32) / np.sqrt(256),
        "bout": np.zeros(128, np.float32),
    }
    out = kernel(**fake)
    print("out", out.shape, out.dtype, float(np.abs(out).mean()))
